# revision 26
# baseline (speedup 1.0000x reference)
"""kNN graph construction (N=4 sets, M=4096 points, D=128, k=16) on 8 trn2 cores.

Sharding: core c handles point set c//2, row half c%2 (2048 query rows x 4096
candidates).  Each core's input set is rotated so its rows come first; the SPMD
program is identical across cores and host code un-rotates returned indices.

Per-core device program:
  - load xT hi/lo fp32r (e8m11) parts [128,4096], split + transposed on host
    (d on partitions); any e8m11-representable split with hi+lo ~= x works
  - per 512-col chunk: one bf16 K=3 matmul (ones3^T @ bias3, where bias3 is
    the host-side 3-term bf16 split of -x2/2, exact to 2^-24) opens the PSUM
    group, then hi.hi + hi.lo + lo.hi fp32r matmuls accumulate on top
    => s[i,j] = x_i . x_j - |x_j|^2/2 at ~fp32 precision, a monotone
    transform of -dist(i,j)
  - ACT evicts PSUM -> SBUF
  - DVE top-16 per row: max per 512-chunk (8x) -> 64 candidates; max /
    match_replace / max on candidates -> rank 1-8 and 9-16 values; two
    full-row max_index calls recover indices (first-match = lowest index,
    matching jax.lax.top_k tie-breaking)
  - DMA idx [128,16] u32 per row-block to DRAM
"""

import os
import sys

import ml_dtypes
import numpy as np

for _p in (os.environ.get("TRN_RL_REPO"), "/opt/trn_rl_repo"):
    if _p and _p not in sys.path and os.path.isdir(_p):
        sys.path.insert(0, _p)

N_SETS = 4
M = 4096
D = 128
K = 16
N_CORES = 8
ROWS_PER_CORE = M // 2  # 2048
ROW_TILES = ROWS_PER_CORE // 128  # 16
CHUNK = 512
N_CHUNKS = M // CHUNK  # 8
NEG_INF = -1.0e30

_compiled = None


def _round_f32r(a):
    """Round f32 to fp32r (e8m11): keep 11 mantissa bits, RNE."""
    bits = np.ascontiguousarray(a, dtype=np.float32).view(np.uint32)
    keep = 12
    rounded = bits + np.uint32((1 << (keep - 1)) - 1) + ((bits >> keep) & 1)
    rounded &= np.uint32(0xFFFFFFFF ^ ((1 << keep) - 1))
    return rounded.view(np.float32)


def build_program():
    import concourse.bacc as bacc
    import concourse.mybir as mybir
    import concourse.tile as tile

    f32 = mybir.dt.float32
    f32r = mybir.dt.float32r
    bf16 = mybir.dt.bfloat16
    u32 = mybir.dt.uint32

    nc = bacc.Bacc(
        "TRN2",
        target_bir_lowering=False,
        debug=False,
        enable_asserts=False,
    )

    xth_in = nc.dram_tensor("xth", [128, M], f32, kind="ExternalInput").ap()
    xtl_in = nc.dram_tensor("xtl", [128, M], f32, kind="ExternalInput").ap()
    negx2_in = nc.dram_tensor("negx2b3", [3, M], mybir.dt.bfloat16, kind="ExternalInput").ap()
    idx_out = nc.dram_tensor(
        "idx_out", [ROWS_PER_CORE, K], u32, kind="ExternalOutput"
    ).ap()

    with tile.TileContext(nc) as tc:
        with tc.tile_pool(name="const", bufs=1) as constp, tc.tile_pool(
            name="ld", bufs=4
        ) as ldp:
            negx2sb = constp.tile([3, M], bf16)
            nc.sync.dma_start(negx2sb[:], negx2_in[:, :])
            ones3 = constp.tile([3, 128], bf16)
            nc.vector.memset(ones3[:], 1.0)
            # chunked load + on-chip f32->f32r rounding copy (values already
            # e8m11 on host, so the ACT copy is exact); chunking lets the
            # first matmuls start as soon as chunk 0 lands
            xhc, xlc = [], []
            for c in range(N_CHUNKS):
                for lst, src_ap, nm in ((xhc, xth_in, "h"), (xlc, xtl_in, "l")):
                    st = ldp.tile([128, CHUNK], f32, tag="stage")
                    eng = nc.sync if c % 2 == 0 else nc.gpsimd
                    eng.dma_start(st[:], src_ap[:, c * CHUNK : (c + 1) * CHUNK])
                    tr = constp.tile([128, CHUNK], f32r, tag=f"x{nm}{c}")
                    nc.scalar.copy(tr[:], st[:])
                    lst.append(tr)

            with tc.tile_pool(name="mm", bufs=4, space="PSUM") as mmp, tc.tile_pool(
                name="sbuf_s", bufs=5
            ) as sp, tc.tile_pool(name="small", bufs=3) as smallp:
                for t in range(ROW_TILES):
                    s_sb = sp.tile([128, M], f32, tag="s")
                    rc, ro = divmod(t, 4)
                    rh = xhc[rc][:, ro * 128 : (ro + 1) * 128]
                    rl = xlc[rc][:, ro * 128 : (ro + 1) * 128]
                    for g in range(4):
                        ps = mmp.tile([128, 1024], f32, tag="ps")
                        for q in range(2):
                            c = g * 2 + q
                            cs = c * CHUNK
                            pslice = ps[:, q * 512 : (q + 1) * 512]
                            nc.tensor.matmul(
                                pslice,
                                lhsT=ones3[:],
                                rhs=negx2sb[:, cs : cs + 512],
                                start=True,
                                stop=False,
                            )
                            ch = xhc[c][:]
                            cl = xlc[c][:]
                            nc.tensor.matmul(
                                pslice, lhsT=rh, rhs=ch, start=False, stop=False
                            )
                            nc.tensor.matmul(
                                pslice, lhsT=rh, rhs=cl, start=False, stop=False
                            )
                            nc.tensor.matmul(
                                pslice, lhsT=rl, rhs=ch, start=False, stop=True
                            )
                        nc.scalar.copy(s_sb[:, g * 1024 : (g + 1) * 1024], ps[:])

                    cand = smallp.tile([128, 8 * N_CHUNKS], f32, tag="cand")
                    for c in range(N_CHUNKS):
                        nc.vector.max(
                            cand[:, c * 8 : (c + 1) * 8],
                            s_sb[:, c * CHUNK : (c + 1) * CHUNK],
                        )
                    f8a = smallp.tile([128, 8], f32, tag="f8a")
                    nc.vector.max(f8a[:], cand[:])
                    cand_mr = smallp.tile([128, 8 * N_CHUNKS], f32, tag="cmr")
                    nc.vector.match_replace(
                        out=cand_mr[:],
                        in_to_replace=f8a[:],
                        in_values=cand[:],
                        imm_value=NEG_INF,
                    )
                    f8b = smallp.tile([128, 8], f32, tag="f8b")
                    nc.vector.max(f8b[:], cand_mr[:])

                    idx16 = smallp.tile([128, K], u32, tag="idx")
                    nc.vector.max_index(idx16[:, 0:8], f8a[:], s_sb[:])
                    nc.vector.max_index(idx16[:, 8:16], f8b[:], s_sb[:])
                    nc.sync.dma_start(
                        idx_out[t * 128 : (t + 1) * 128, :], idx16[:]
                    )

    nc.compile()
    return nc


def get_program():
    global _compiled
    if _compiled is None:
        _compiled = build_program()
    return _compiled


def make_in_maps(x):
    """x: [N_SETS, M, D] float32 -> list of 8 per-core input dicts."""
    x = np.asarray(x, dtype=np.float32)
    in_maps = []
    for c in range(N_CORES):
        s, half = divmod(c, 2)
        xs = x[s]
        if half:
            xs = np.concatenate([xs[ROWS_PER_CORE:], xs[:ROWS_PER_CORE]], axis=0)
        xs = np.ascontiguousarray(xs)
        x2 = np.einsum("md,md->m", xs, xs, dtype=np.float32).astype(np.float32)
        b = (-0.5 * x2).astype(np.float32)
        b3 = np.zeros((3, M), dtype=ml_dtypes.bfloat16)
        rem = b
        for i in range(3):
            b3[i] = rem.astype(ml_dtypes.bfloat16)
            rem = rem - b3[i].astype(np.float32)
        xt = np.ascontiguousarray(xs.T)
        xth = _round_f32r(xt)
        xtl = _round_f32r(xt - xth)
        in_maps.append({"xth": xth, "xtl": xtl, "negx2b3": b3})
    return in_maps


def assemble(per_core_idx):
    """per_core_idx: list of 8 [2048,16] u32 arrays -> (src, dst) int32."""
    src = np.empty((N_SETS, M, K), dtype=np.int64)
    for c in range(N_CORES):
        s, half = divmod(c, 2)
        idx = per_core_idx[c].astype(np.int64)
        local = (idx + half * ROWS_PER_CORE) % M
        src[s, half * ROWS_PER_CORE : (half + 1) * ROWS_PER_CORE, :] = local + s * M
    src = src.reshape(-1).astype(np.int32)
    dst = np.repeat(np.arange(N_SETS * M, dtype=np.int32), K)
    return src, dst


def run_spmd(x, trace=False, **kwargs):
    from concourse import bass_utils

    nc = get_program()
    in_maps = make_in_maps(x)
    res = bass_utils.run_bass_kernel_spmd(
        nc, in_maps, core_ids=list(range(N_CORES)), trace=trace, **kwargs
    )
    per_core = [res.results[c]["idx_out"] for c in range(N_CORES)]
    return assemble(per_core), res


def kernel(x, k):
    k = int(np.asarray(k))
    assert k == K, f"kernel hardcoded for k={K}, got {k}"
    x = np.asarray(x, dtype=np.float32)
    assert x.shape == (N_SETS, M, D), f"unexpected shape {x.shape}"
    (src, dst), _ = run_spmd(x)
    return src, dst


# revision 27
# speedup vs baseline: 1.1874x; 1.1874x over previous
"""kNN graph construction (N=4 sets, M=4096 points, D=128, k=16) on 8 trn2 cores.

Sharding: core c handles point set c//2, row half c%2 (2048 query rows x 4096
candidates).  Each core's input set is rotated so its rows come first; the SPMD
program is identical across cores and host code un-rotates returned indices.

Per-core device program:
  - load xT hi/lo fp32r (e8m11) parts [128,4096], split + transposed on host
    (d on partitions); any e8m11-representable split with hi+lo ~= x works
  - per 512-col chunk: one bf16 K=3 matmul (ones3^T @ bias3, where bias3 is
    the host-side 3-term bf16 split of -x2/2, exact to 2^-24) opens the PSUM
    group, then hi.hi + hi.lo + lo.hi fp32r matmuls accumulate on top
    => s[i,j] = x_i . x_j - |x_j|^2/2 at ~fp32 precision, a monotone
    transform of -dist(i,j)
  - ACT evicts PSUM -> SBUF
  - DVE top-16 per row: max per 512-chunk (8x) -> 64 candidates; max /
    match_replace / max on candidates -> rank 1-8 and 9-16 values; two
    full-row max_index calls recover indices (first-match = lowest index,
    matching jax.lax.top_k tie-breaking)
  - DMA idx [128,16] u32 per row-block to DRAM
"""

import os
import sys

import ml_dtypes
import numpy as np

for _p in (os.environ.get("TRN_RL_REPO"), "/opt/trn_rl_repo"):
    if _p and _p not in sys.path and os.path.isdir(_p):
        sys.path.insert(0, _p)

N_SETS = 4
M = 4096
D = 128
K = 16
N_CORES = 8
ROWS_PER_CORE = M // 2  # 2048
ROW_TILES = ROWS_PER_CORE // 128  # 16
CHUNK = 512
N_CHUNKS = M // CHUNK  # 8
NEG_INF = -1.0e30

_compiled = None


def _round_f32r(a):
    """Round f32 to fp32r (e8m11): keep 11 mantissa bits, RNE."""
    bits = np.ascontiguousarray(a, dtype=np.float32).view(np.uint32)
    keep = 12
    rounded = bits + np.uint32((1 << (keep - 1)) - 1) + ((bits >> keep) & 1)
    rounded &= np.uint32(0xFFFFFFFF ^ ((1 << keep) - 1))
    return rounded.view(np.float32)


def build_program():
    import concourse.bacc as bacc
    import concourse.mybir as mybir
    import concourse.tile as tile

    f32 = mybir.dt.float32
    f32r = mybir.dt.float32r
    bf16 = mybir.dt.bfloat16
    u32 = mybir.dt.uint32

    nc = bacc.Bacc(
        "TRN2",
        target_bir_lowering=False,
        debug=False,
        enable_asserts=False,
    )

    xth_in = nc.dram_tensor("xth", [128, M], f32, kind="ExternalInput").ap()
    xtl_in = nc.dram_tensor("xtl", [128, M], f32, kind="ExternalInput").ap()
    negx2_in = nc.dram_tensor("negx2b3", [3, M], mybir.dt.bfloat16, kind="ExternalInput").ap()
    idx_out = nc.dram_tensor(
        "idx_out", [ROWS_PER_CORE, K], u32, kind="ExternalOutput"
    ).ap()

    with tile.TileContext(nc) as tc:
        with tc.tile_pool(name="const", bufs=1) as constp, tc.tile_pool(
            name="ld", bufs=4
        ) as ldp:
            negx2sb = constp.tile([3, M], bf16)
            nc.sync.dma_start(negx2sb[:], negx2_in[:, :])
            ones3 = constp.tile([3, 128], bf16)
            nc.vector.memset(ones3[:], 1.0)
            # chunked load + on-chip f32->f32r rounding copy (values already
            # e8m11 on host, so the ACT copy is exact); chunking lets the
            # first matmuls start as soon as chunk 0 lands
            xhc, xlc = [], []
            for c in range(N_CHUNKS):
                for lst, src_ap, nm in ((xhc, xth_in, "h"), (xlc, xtl_in, "l")):
                    st = ldp.tile([128, CHUNK], f32, tag="stage")
                    eng = nc.sync if c % 2 == 0 else nc.gpsimd
                    eng.dma_start(st[:], src_ap[:, c * CHUNK : (c + 1) * CHUNK])
                    tr = constp.tile([128, CHUNK], f32r, tag=f"x{nm}{c}")
                    nc.scalar.copy(tr[:], st[:])
                    lst.append(tr)

            with tc.tile_pool(name="mm", bufs=4, space="PSUM") as mmp, tc.tile_pool(
                name="sbuf_s", bufs=4
            ) as sp, tc.tile_pool(name="small", bufs=3) as smallp:
                for t in range(ROW_TILES):
                    s_sb = sp.tile([128, M], f32, tag="s")
                    rc, ro = divmod(t, 4)
                    rh = xhc[rc][:, ro * 128 : (ro + 1) * 128]
                    rl = xlc[rc][:, ro * 128 : (ro + 1) * 128]
                    for g in range(4):
                        ps = mmp.tile([128, 1024], f32, tag="ps")
                        for q in range(2):
                            c = g * 2 + q
                            cs = c * CHUNK
                            pslice = ps[:, q * 512 : (q + 1) * 512]
                            nc.tensor.matmul(
                                pslice,
                                lhsT=ones3[:],
                                rhs=negx2sb[:, cs : cs + 512],
                                start=True,
                                stop=False,
                            )
                            ch = xhc[c][:]
                            cl = xlc[c][:]
                            nc.tensor.matmul(
                                pslice, lhsT=rh, rhs=ch, start=False, stop=False
                            )
                            nc.tensor.matmul(
                                pslice, lhsT=rh, rhs=cl, start=False, stop=False
                            )
                            nc.tensor.matmul(
                                pslice, lhsT=rl, rhs=ch, start=False, stop=True
                            )
                        nc.scalar.copy(s_sb[:, g * 1024 : (g + 1) * 1024], ps[:])

                    cand = smallp.tile([128, 8 * N_CHUNKS], f32, tag="cand")
                    for c in range(N_CHUNKS):
                        nc.vector.max(
                            cand[:, c * 8 : (c + 1) * 8],
                            s_sb[:, c * CHUNK : (c + 1) * CHUNK],
                        )
                    f8a = smallp.tile([128, 8], f32, tag="f8a")
                    nc.vector.max(f8a[:], cand[:])
                    cand_mr = smallp.tile([128, 8 * N_CHUNKS], f32, tag="cmr")
                    nc.vector.match_replace(
                        out=cand_mr[:],
                        in_to_replace=f8a[:],
                        in_values=cand[:],
                        imm_value=NEG_INF,
                    )
                    f8b = smallp.tile([128, 8], f32, tag="f8b")
                    nc.vector.max(f8b[:], cand_mr[:])

                    idx16 = smallp.tile([128, K], u32, tag="idx")
                    nc.vector.max_index(idx16[:, 0:8], f8a[:], s_sb[:])
                    nc.vector.max_index(idx16[:, 8:16], f8b[:], s_sb[:])
                    nc.sync.dma_start(
                        idx_out[t * 128 : (t + 1) * 128, :], idx16[:]
                    )

    nc.compile()
    return nc


def get_program():
    global _compiled
    if _compiled is None:
        _compiled = build_program()
    return _compiled


def make_in_maps(x):
    """x: [N_SETS, M, D] float32 -> list of 8 per-core input dicts."""
    x = np.asarray(x, dtype=np.float32)
    in_maps = []
    for c in range(N_CORES):
        s, half = divmod(c, 2)
        xs = x[s]
        if half:
            xs = np.concatenate([xs[ROWS_PER_CORE:], xs[:ROWS_PER_CORE]], axis=0)
        xs = np.ascontiguousarray(xs)
        x2 = np.einsum("md,md->m", xs, xs, dtype=np.float32).astype(np.float32)
        b = (-0.5 * x2).astype(np.float32)
        b3 = np.zeros((3, M), dtype=ml_dtypes.bfloat16)
        rem = b
        for i in range(3):
            b3[i] = rem.astype(ml_dtypes.bfloat16)
            rem = rem - b3[i].astype(np.float32)
        xt = np.ascontiguousarray(xs.T)
        xth = _round_f32r(xt)
        xtl = _round_f32r(xt - xth)
        in_maps.append({"xth": xth, "xtl": xtl, "negx2b3": b3})
    return in_maps


def assemble(per_core_idx):
    """per_core_idx: list of 8 [2048,16] u32 arrays -> (src, dst) int32."""
    src = np.empty((N_SETS, M, K), dtype=np.int64)
    for c in range(N_CORES):
        s, half = divmod(c, 2)
        idx = per_core_idx[c].astype(np.int64)
        local = (idx + half * ROWS_PER_CORE) % M
        src[s, half * ROWS_PER_CORE : (half + 1) * ROWS_PER_CORE, :] = local + s * M
    src = src.reshape(-1).astype(np.int32)
    dst = np.repeat(np.arange(N_SETS * M, dtype=np.int32), K)
    return src, dst


def run_spmd(x, trace=False, **kwargs):
    from concourse import bass_utils

    nc = get_program()
    in_maps = make_in_maps(x)
    res = bass_utils.run_bass_kernel_spmd(
        nc, in_maps, core_ids=list(range(N_CORES)), trace=trace, **kwargs
    )
    per_core = [res.results[c]["idx_out"] for c in range(N_CORES)]
    return assemble(per_core), res


def kernel(x, k):
    k = int(np.asarray(k))
    assert k == K, f"kernel hardcoded for k={K}, got {k}"
    x = np.asarray(x, dtype=np.float32)
    assert x.shape == (N_SETS, M, D), f"unexpected shape {x.shape}"
    (src, dst), _ = run_spmd(x)
    return src, dst


# revision 28
# speedup vs baseline: 1.1902x; 1.0024x over previous
"""kNN graph construction (N=4 sets, M=4096 points, D=128, k=16) on 8 trn2 cores.

Sharding: core c handles point set c//2, row half c%2 (2048 query rows x 4096
candidates).  Each core's input set is rotated so its rows come first; the SPMD
program is identical across cores and host code un-rotates returned indices.

Per-core device program:
  - load xT hi/lo fp32r (e8m11) parts [128,4096], split + transposed on host
    (d on partitions); any e8m11-representable split with hi+lo ~= x works
  - per 512-col chunk: one bf16 K=3 matmul (ones3^T @ bias3, where bias3 is
    the host-side 3-term bf16 split of -x2/2, exact to 2^-24) opens the PSUM
    group, then hi.hi + hi.lo + lo.hi fp32r matmuls accumulate on top
    => s[i,j] = x_i . x_j - |x_j|^2/2 at ~fp32 precision, a monotone
    transform of -dist(i,j)
  - ACT evicts PSUM -> SBUF
  - DVE top-16 per row: max per 512-chunk (8x) -> 64 candidates; max /
    match_replace / max on candidates -> rank 1-8 and 9-16 values; two
    full-row max_index calls recover indices (first-match = lowest index,
    matching jax.lax.top_k tie-breaking)
  - DMA idx [128,16] u32 per row-block to DRAM
"""

import os
import sys

import ml_dtypes
import numpy as np

for _p in (os.environ.get("TRN_RL_REPO"), "/opt/trn_rl_repo"):
    if _p and _p not in sys.path and os.path.isdir(_p):
        sys.path.insert(0, _p)

N_SETS = 4
M = 4096
D = 128
K = 16
N_CORES = 8
ROWS_PER_CORE = M // 2  # 2048
ROW_TILES = ROWS_PER_CORE // 128  # 16
CHUNK = 512
N_CHUNKS = M // CHUNK  # 8
NEG_INF = -1.0e30

_compiled = None


def _round_f32r(a):
    """Round f32 to fp32r (e8m11): keep 11 mantissa bits, RNE."""
    bits = np.ascontiguousarray(a, dtype=np.float32).view(np.uint32)
    keep = 12
    rounded = bits + np.uint32((1 << (keep - 1)) - 1) + ((bits >> keep) & 1)
    rounded &= np.uint32(0xFFFFFFFF ^ ((1 << keep) - 1))
    return rounded.view(np.float32)


def build_program():
    import concourse.bacc as bacc
    import concourse.mybir as mybir
    import concourse.tile as tile

    f32 = mybir.dt.float32
    f32r = mybir.dt.float32r
    bf16 = mybir.dt.bfloat16
    u32 = mybir.dt.uint32

    nc = bacc.Bacc(
        "TRN2",
        target_bir_lowering=False,
        debug=False,
        enable_asserts=False,
    )

    xth_in = nc.dram_tensor("xth", [128, M], f32, kind="ExternalInput").ap()
    xtl_in = nc.dram_tensor("xtl", [128, M], f32, kind="ExternalInput").ap()
    negx2_in = nc.dram_tensor("negx2b3", [3, M], mybir.dt.bfloat16, kind="ExternalInput").ap()
    idx_out = nc.dram_tensor(
        "idx_out", [ROWS_PER_CORE, K], u32, kind="ExternalOutput"
    ).ap()

    with tile.TileContext(nc) as tc:
        with tc.tile_pool(name="const", bufs=1) as constp, tc.tile_pool(
            name="ld", bufs=4
        ) as ldp:
            negx2sb = constp.tile([3, M], bf16)
            nc.sync.dma_start(negx2sb[:], negx2_in[:, :])
            ones3 = constp.tile([3, 128], bf16)
            nc.vector.memset(ones3[:], 1.0)
            # PE pstate warmup while the first DMA chunks land
            with tc.tile_pool(name="warm", bufs=1, space="PSUM") as warmp:
                wps = warmp.tile([128, 128], f32)
                for _ in range(10):
                    nc.tensor.matmul(
                        wps[:], lhsT=ones3[:], rhs=ones3[:], start=True, stop=True
                    )
            # chunked load + on-chip f32->f32r rounding copy (values already
            # e8m11 on host, so the ACT copy is exact); chunking lets the
            # first matmuls start as soon as chunk 0 lands
            xhc, xlc = [], []
            for c in range(N_CHUNKS):
                for lst, src_ap, nm in ((xhc, xth_in, "h"), (xlc, xtl_in, "l")):
                    st = ldp.tile([128, CHUNK], f32, tag="stage")
                    eng = nc.sync if c % 2 == 0 else nc.gpsimd
                    eng.dma_start(st[:], src_ap[:, c * CHUNK : (c + 1) * CHUNK])
                    tr = constp.tile([128, CHUNK], f32r, tag=f"x{nm}{c}")
                    nc.scalar.copy(tr[:], st[:])
                    lst.append(tr)

            with tc.tile_pool(name="mm", bufs=4, space="PSUM") as mmp, tc.tile_pool(
                name="sbuf_s", bufs=4
            ) as sp, tc.tile_pool(name="small", bufs=3) as smallp:
                for t in range(ROW_TILES):
                    s_sb = sp.tile([128, M], f32, tag="s")
                    rc, ro = divmod(t, 4)
                    rh = xhc[rc][:, ro * 128 : (ro + 1) * 128]
                    rl = xlc[rc][:, ro * 128 : (ro + 1) * 128]
                    for g in range(4):
                        ps = mmp.tile([128, 1024], f32, tag="ps")
                        for q in range(2):
                            c = g * 2 + q
                            cs = c * CHUNK
                            pslice = ps[:, q * 512 : (q + 1) * 512]
                            nc.tensor.matmul(
                                pslice,
                                lhsT=ones3[:],
                                rhs=negx2sb[:, cs : cs + 512],
                                start=True,
                                stop=False,
                            )
                            ch = xhc[c][:]
                            cl = xlc[c][:]
                            nc.tensor.matmul(
                                pslice, lhsT=rh, rhs=ch, start=False, stop=False
                            )
                            nc.tensor.matmul(
                                pslice, lhsT=rh, rhs=cl, start=False, stop=False
                            )
                            nc.tensor.matmul(
                                pslice, lhsT=rl, rhs=ch, start=False, stop=True
                            )
                        nc.scalar.copy(s_sb[:, g * 1024 : (g + 1) * 1024], ps[:])

                    cand = smallp.tile([128, 8 * N_CHUNKS], f32, tag="cand")
                    for c in range(N_CHUNKS):
                        nc.vector.max(
                            cand[:, c * 8 : (c + 1) * 8],
                            s_sb[:, c * CHUNK : (c + 1) * CHUNK],
                        )
                    f8a = smallp.tile([128, 8], f32, tag="f8a")
                    nc.vector.max(f8a[:], cand[:])
                    cand_mr = smallp.tile([128, 8 * N_CHUNKS], f32, tag="cmr")
                    nc.vector.match_replace(
                        out=cand_mr[:],
                        in_to_replace=f8a[:],
                        in_values=cand[:],
                        imm_value=NEG_INF,
                    )
                    f8b = smallp.tile([128, 8], f32, tag="f8b")
                    nc.vector.max(f8b[:], cand_mr[:])

                    idx16 = smallp.tile([128, K], u32, tag="idx")
                    nc.vector.max_index(idx16[:, 0:8], f8a[:], s_sb[:])
                    nc.vector.max_index(idx16[:, 8:16], f8b[:], s_sb[:])
                    nc.sync.dma_start(
                        idx_out[t * 128 : (t + 1) * 128, :], idx16[:]
                    )

    nc.compile()
    return nc


def get_program():
    global _compiled
    if _compiled is None:
        _compiled = build_program()
    return _compiled


def make_in_maps(x):
    """x: [N_SETS, M, D] float32 -> list of 8 per-core input dicts."""
    x = np.asarray(x, dtype=np.float32)
    in_maps = []
    for c in range(N_CORES):
        s, half = divmod(c, 2)
        xs = x[s]
        if half:
            xs = np.concatenate([xs[ROWS_PER_CORE:], xs[:ROWS_PER_CORE]], axis=0)
        xs = np.ascontiguousarray(xs)
        x2 = np.einsum("md,md->m", xs, xs, dtype=np.float32).astype(np.float32)
        b = (-0.5 * x2).astype(np.float32)
        b3 = np.zeros((3, M), dtype=ml_dtypes.bfloat16)
        rem = b
        for i in range(3):
            b3[i] = rem.astype(ml_dtypes.bfloat16)
            rem = rem - b3[i].astype(np.float32)
        xt = np.ascontiguousarray(xs.T)
        xth = _round_f32r(xt)
        xtl = _round_f32r(xt - xth)
        in_maps.append({"xth": xth, "xtl": xtl, "negx2b3": b3})
    return in_maps


def assemble(per_core_idx):
    """per_core_idx: list of 8 [2048,16] u32 arrays -> (src, dst) int32."""
    src = np.empty((N_SETS, M, K), dtype=np.int64)
    for c in range(N_CORES):
        s, half = divmod(c, 2)
        idx = per_core_idx[c].astype(np.int64)
        local = (idx + half * ROWS_PER_CORE) % M
        src[s, half * ROWS_PER_CORE : (half + 1) * ROWS_PER_CORE, :] = local + s * M
    src = src.reshape(-1).astype(np.int32)
    dst = np.repeat(np.arange(N_SETS * M, dtype=np.int32), K)
    return src, dst


def run_spmd(x, trace=False, **kwargs):
    from concourse import bass_utils

    nc = get_program()
    in_maps = make_in_maps(x)
    res = bass_utils.run_bass_kernel_spmd(
        nc, in_maps, core_ids=list(range(N_CORES)), trace=trace, **kwargs
    )
    per_core = [res.results[c]["idx_out"] for c in range(N_CORES)]
    return assemble(per_core), res


def kernel(x, k):
    k = int(np.asarray(k))
    assert k == K, f"kernel hardcoded for k={K}, got {k}"
    x = np.asarray(x, dtype=np.float32)
    assert x.shape == (N_SETS, M, D), f"unexpected shape {x.shape}"
    (src, dst), _ = run_spmd(x)
    return src, dst


# revision 29
# speedup vs baseline: 1.2154x; 1.0212x over previous
"""kNN graph construction (N=4 sets, M=4096 points, D=128, k=16) on 8 trn2 cores.

Sharding: core c handles point set c//2, row half c%2 (2048 query rows x 4096
candidates).  Each core's input set is rotated so its rows come first; the SPMD
program is identical across cores and host code un-rotates returned indices.

Per-core device program:
  - load xT hi/lo fp32r (e8m11) parts [128,4096], split + transposed on host
    (d on partitions); any e8m11-representable split with hi+lo ~= x works
  - per 512-col chunk: one bf16 K=3 matmul (ones3^T @ bias3, where bias3 is
    the host-side 3-term bf16 split of -x2/2, exact to 2^-24) opens the PSUM
    group, then hi.hi + hi.lo + lo.hi fp32r matmuls accumulate on top
    => s[i,j] = x_i . x_j - |x_j|^2/2 at ~fp32 precision, a monotone
    transform of -dist(i,j)
  - ACT evicts PSUM -> SBUF
  - DVE top-16 per row: max per 512-chunk (8x) -> 64 candidates; max /
    match_replace / max on candidates -> rank 1-8 and 9-16 values; two
    full-row max_index calls recover indices (first-match = lowest index,
    matching jax.lax.top_k tie-breaking)
  - DMA idx [128,16] u32 per row-block to DRAM
"""

import os
import sys

import ml_dtypes
import numpy as np

for _p in (os.environ.get("TRN_RL_REPO"), "/opt/trn_rl_repo"):
    if _p and _p not in sys.path and os.path.isdir(_p):
        sys.path.insert(0, _p)

N_SETS = 4
M = 4096
D = 128
K = 16
N_CORES = 8
ROWS_PER_CORE = M // 2  # 2048
ROW_TILES = ROWS_PER_CORE // 128  # 16
CHUNK = 512
N_CHUNKS = M // CHUNK  # 8
NEG_INF = -1.0e30

_compiled = None


def _round_f32r(a):
    """Round f32 to fp32r (e8m11): keep 11 mantissa bits, RNE."""
    bits = np.ascontiguousarray(a, dtype=np.float32).view(np.uint32)
    keep = 12
    rounded = bits + np.uint32((1 << (keep - 1)) - 1) + ((bits >> keep) & 1)
    rounded &= np.uint32(0xFFFFFFFF ^ ((1 << keep) - 1))
    return rounded.view(np.float32)


def build_program():
    import concourse.bacc as bacc
    import concourse.mybir as mybir
    import concourse.tile as tile

    f32 = mybir.dt.float32
    f32r = mybir.dt.float32r
    bf16 = mybir.dt.bfloat16
    u32 = mybir.dt.uint32

    nc = bacc.Bacc(
        "TRN2",
        target_bir_lowering=False,
        debug=False,
        enable_asserts=False,
    )

    xth_in = nc.dram_tensor("xth", [128, M], f32, kind="ExternalInput").ap()
    xtl_in = nc.dram_tensor("xtl", [128, M], f32, kind="ExternalInput").ap()
    negx2_in = nc.dram_tensor("negx2b3", [3, M], mybir.dt.bfloat16, kind="ExternalInput").ap()
    idx_out = nc.dram_tensor(
        "idx_out", [ROWS_PER_CORE, K], u32, kind="ExternalOutput"
    ).ap()
    warm_out = nc.dram_tensor("warm_out", [1, 1], f32, kind="ExternalOutput").ap()

    with tile.TileContext(nc) as tc:
        with tc.tile_pool(name="const", bufs=1) as constp, tc.tile_pool(
            name="ld", bufs=4
        ) as ldp:
            negx2sb = constp.tile([3, M], bf16)
            nc.sync.dma_start(negx2sb[:], negx2_in[:, :])
            ones3 = constp.tile([3, 128], bf16)
            nc.vector.memset(ones3[:], 1.0)
            # PE pstate warmup while the first DMA chunks land; the dummy
            # output keeps it from being dead-code-eliminated
            with tc.tile_pool(name="warm", bufs=1, space="PSUM") as warmp:
                wps = warmp.tile([128, 128], f32)
                for _ in range(10):
                    nc.tensor.matmul(
                        wps[:], lhsT=ones3[:], rhs=ones3[:], start=True, stop=True
                    )
                wsb = constp.tile([1, 1], f32)
                nc.scalar.copy(wsb[:], wps[0:1, 0:1])
                nc.sync.dma_start(warm_out[:, :], wsb[:])
            # chunked load + on-chip f32->f32r rounding copy (values already
            # e8m11 on host, so the ACT copy is exact); chunking lets the
            # first matmuls start as soon as chunk 0 lands
            xhc, xlc = [], []
            for c in range(N_CHUNKS):
                for lst, src_ap, nm in ((xhc, xth_in, "h"), (xlc, xtl_in, "l")):
                    st = ldp.tile([128, CHUNK], f32, tag="stage")
                    eng = nc.sync if c % 2 == 0 else nc.gpsimd
                    eng.dma_start(st[:], src_ap[:, c * CHUNK : (c + 1) * CHUNK])
                    tr = constp.tile([128, CHUNK], f32r, tag=f"x{nm}{c}")
                    nc.scalar.copy(tr[:], st[:])
                    lst.append(tr)

            with tc.tile_pool(name="mm", bufs=4, space="PSUM") as mmp, tc.tile_pool(
                name="sbuf_s", bufs=4
            ) as sp, tc.tile_pool(name="small", bufs=3) as smallp:
                for t in range(ROW_TILES):
                    s_sb = sp.tile([128, M], f32, tag="s")
                    rc, ro = divmod(t, 4)
                    rh = xhc[rc][:, ro * 128 : (ro + 1) * 128]
                    rl = xlc[rc][:, ro * 128 : (ro + 1) * 128]
                    for g in range(4):
                        ps = mmp.tile([128, 1024], f32, tag="ps")
                        for q in range(2):
                            c = g * 2 + q
                            cs = c * CHUNK
                            pslice = ps[:, q * 512 : (q + 1) * 512]
                            nc.tensor.matmul(
                                pslice,
                                lhsT=ones3[:],
                                rhs=negx2sb[:, cs : cs + 512],
                                start=True,
                                stop=False,
                            )
                            ch = xhc[c][:]
                            cl = xlc[c][:]
                            nc.tensor.matmul(
                                pslice, lhsT=rh, rhs=ch, start=False, stop=False
                            )
                            nc.tensor.matmul(
                                pslice, lhsT=rh, rhs=cl, start=False, stop=False
                            )
                            nc.tensor.matmul(
                                pslice, lhsT=rl, rhs=ch, start=False, stop=True
                            )
                        nc.scalar.copy(s_sb[:, g * 1024 : (g + 1) * 1024], ps[:])

                    cand = smallp.tile([128, 8 * N_CHUNKS], f32, tag="cand")
                    for c in range(N_CHUNKS):
                        nc.vector.max(
                            cand[:, c * 8 : (c + 1) * 8],
                            s_sb[:, c * CHUNK : (c + 1) * CHUNK],
                        )
                    f8a = smallp.tile([128, 8], f32, tag="f8a")
                    nc.vector.max(f8a[:], cand[:])
                    cand_mr = smallp.tile([128, 8 * N_CHUNKS], f32, tag="cmr")
                    nc.vector.match_replace(
                        out=cand_mr[:],
                        in_to_replace=f8a[:],
                        in_values=cand[:],
                        imm_value=NEG_INF,
                    )
                    f8b = smallp.tile([128, 8], f32, tag="f8b")
                    nc.vector.max(f8b[:], cand_mr[:])

                    idx16 = smallp.tile([128, K], u32, tag="idx")
                    nc.vector.max_index(idx16[:, 0:8], f8a[:], s_sb[:])
                    nc.vector.max_index(idx16[:, 8:16], f8b[:], s_sb[:])
                    nc.sync.dma_start(
                        idx_out[t * 128 : (t + 1) * 128, :], idx16[:]
                    )

    nc.compile()
    return nc


def get_program():
    global _compiled
    if _compiled is None:
        _compiled = build_program()
    return _compiled


def make_in_maps(x):
    """x: [N_SETS, M, D] float32 -> list of 8 per-core input dicts."""
    x = np.asarray(x, dtype=np.float32)
    in_maps = []
    for c in range(N_CORES):
        s, half = divmod(c, 2)
        xs = x[s]
        if half:
            xs = np.concatenate([xs[ROWS_PER_CORE:], xs[:ROWS_PER_CORE]], axis=0)
        xs = np.ascontiguousarray(xs)
        x2 = np.einsum("md,md->m", xs, xs, dtype=np.float32).astype(np.float32)
        b = (-0.5 * x2).astype(np.float32)
        b3 = np.zeros((3, M), dtype=ml_dtypes.bfloat16)
        rem = b
        for i in range(3):
            b3[i] = rem.astype(ml_dtypes.bfloat16)
            rem = rem - b3[i].astype(np.float32)
        xt = np.ascontiguousarray(xs.T)
        xth = _round_f32r(xt)
        xtl = _round_f32r(xt - xth)
        in_maps.append({"xth": xth, "xtl": xtl, "negx2b3": b3})
    return in_maps


def assemble(per_core_idx):
    """per_core_idx: list of 8 [2048,16] u32 arrays -> (src, dst) int32."""
    src = np.empty((N_SETS, M, K), dtype=np.int64)
    for c in range(N_CORES):
        s, half = divmod(c, 2)
        idx = per_core_idx[c].astype(np.int64)
        local = (idx + half * ROWS_PER_CORE) % M
        src[s, half * ROWS_PER_CORE : (half + 1) * ROWS_PER_CORE, :] = local + s * M
    src = src.reshape(-1).astype(np.int32)
    dst = np.repeat(np.arange(N_SETS * M, dtype=np.int32), K)
    return src, dst


def run_spmd(x, trace=False, **kwargs):
    from concourse import bass_utils

    nc = get_program()
    in_maps = make_in_maps(x)
    res = bass_utils.run_bass_kernel_spmd(
        nc, in_maps, core_ids=list(range(N_CORES)), trace=trace, **kwargs
    )
    per_core = [res.results[c]["idx_out"] for c in range(N_CORES)]
    return assemble(per_core), res


def kernel(x, k):
    k = int(np.asarray(k))
    assert k == K, f"kernel hardcoded for k={K}, got {k}"
    x = np.asarray(x, dtype=np.float32)
    assert x.shape == (N_SETS, M, D), f"unexpected shape {x.shape}"
    (src, dst), _ = run_spmd(x)
    return src, dst


# revision 30
# speedup vs baseline: 1.2175x; 1.0017x over previous
"""kNN graph construction (N=4 sets, M=4096 points, D=128, k=16) on 8 trn2 cores.

Sharding: core c handles point set c//2, row half c%2 (2048 query rows x 4096
candidates).  Each core's input set is rotated so its rows come first; the SPMD
program is identical across cores and host code un-rotates returned indices.

Per-core device program:
  - load xT hi/lo fp32r (e8m11) parts [128,4096], split + transposed on host
    (d on partitions); any e8m11-representable split with hi+lo ~= x works
  - per 512-col chunk: one bf16 K=3 matmul (ones3^T @ bias3, where bias3 is
    the host-side 3-term bf16 split of -x2/2, exact to 2^-24) opens the PSUM
    group, then hi.hi + hi.lo + lo.hi fp32r matmuls accumulate on top
    => s[i,j] = x_i . x_j - |x_j|^2/2 at ~fp32 precision, a monotone
    transform of -dist(i,j)
  - ACT evicts PSUM -> SBUF
  - DVE top-16 per row: max per 512-chunk (8x) -> 64 candidates; max /
    match_replace / max on candidates -> rank 1-8 and 9-16 values; two
    full-row max_index calls recover indices (first-match = lowest index,
    matching jax.lax.top_k tie-breaking)
  - DMA idx [128,16] u32 per row-block to DRAM
"""

import os
import sys

import ml_dtypes
import numpy as np

for _p in (os.environ.get("TRN_RL_REPO"), "/opt/trn_rl_repo"):
    if _p and _p not in sys.path and os.path.isdir(_p):
        sys.path.insert(0, _p)

N_SETS = 4
M = 4096
D = 128
K = 16
N_CORES = 8
ROWS_PER_CORE = M // 2  # 2048
ROW_TILES = ROWS_PER_CORE // 128  # 16
CHUNK = 512
N_CHUNKS = M // CHUNK  # 8
NEG_INF = -1.0e30

_compiled = None


def _round_f32r(a):
    """Round f32 to fp32r (e8m11): keep 11 mantissa bits, RNE."""
    bits = np.ascontiguousarray(a, dtype=np.float32).view(np.uint32)
    keep = 12
    rounded = bits + np.uint32((1 << (keep - 1)) - 1) + ((bits >> keep) & 1)
    rounded &= np.uint32(0xFFFFFFFF ^ ((1 << keep) - 1))
    return rounded.view(np.float32)


def build_program():
    import concourse.bacc as bacc
    import concourse.mybir as mybir
    import concourse.tile as tile

    f32 = mybir.dt.float32
    f32r = mybir.dt.float32r
    bf16 = mybir.dt.bfloat16
    u32 = mybir.dt.uint32

    nc = bacc.Bacc(
        "TRN2",
        target_bir_lowering=False,
        debug=False,
        enable_asserts=False,
    )

    xth_in = nc.dram_tensor("xth", [128, M], f32, kind="ExternalInput").ap()
    xtl_in = nc.dram_tensor("xtl", [128, M], f32, kind="ExternalInput").ap()
    negx2_in = nc.dram_tensor("negx2b3", [3, M], mybir.dt.bfloat16, kind="ExternalInput").ap()
    idx_out = nc.dram_tensor(
        "idx_out", [ROWS_PER_CORE, K], u32, kind="ExternalOutput"
    ).ap()
    warm_out = nc.dram_tensor("warm_out", [1, 1], f32, kind="ExternalOutput").ap()

    with tile.TileContext(nc) as tc:
        with tc.tile_pool(name="const", bufs=1) as constp, tc.tile_pool(
            name="ld", bufs=4
        ) as ldp:
            negx2sb = constp.tile([3, M], bf16)
            nc.sync.dma_start(negx2sb[:], negx2_in[:, :])
            ones3 = constp.tile([3, 128], bf16)
            nc.vector.memset(ones3[:], 1.0)
            # PE pstate warmup while the first DMA chunks land; the dummy
            # output keeps it from being dead-code-eliminated
            with tc.tile_pool(name="warm", bufs=1, space="PSUM") as warmp:
                wps = warmp.tile([128, 128], f32)
                for _ in range(10):
                    nc.tensor.matmul(
                        wps[:], lhsT=ones3[:], rhs=ones3[:], start=True, stop=True
                    )
                wsb = constp.tile([1, 1], f32)
                nc.scalar.copy(wsb[:], wps[0:1, 0:1])
                nc.sync.dma_start(warm_out[:, :], wsb[:])
            # chunked load + on-chip f32->f32r rounding copy (values already
            # e8m11 on host, so the ACT copy is exact); chunking lets the
            # first matmuls start as soon as chunk 0 lands
            xhc, xlc = [], []
            for c in range(N_CHUNKS):
                for lst, src_ap, nm in ((xhc, xth_in, "h"), (xlc, xtl_in, "l")):
                    st = ldp.tile([128, CHUNK], f32, tag="stage")
                    eng = nc.sync if c % 2 == 0 else nc.gpsimd
                    eng.dma_start(st[:], src_ap[:, c * CHUNK : (c + 1) * CHUNK])
                    tr = constp.tile([128, CHUNK], f32r, tag=f"x{nm}{c}")
                    nc.scalar.copy(tr[:], st[:])
                    lst.append(tr)

            with tc.tile_pool(name="mm", bufs=8, space="PSUM") as mmp, tc.tile_pool(
                name="sbuf_s", bufs=4
            ) as sp, tc.tile_pool(name="small", bufs=3) as smallp:
                for t in range(ROW_TILES):
                    s_sb = sp.tile([128, M], f32, tag="s")
                    rc, ro = divmod(t, 4)
                    rh = xhc[rc][:, ro * 128 : (ro + 1) * 128]
                    rl = xlc[rc][:, ro * 128 : (ro + 1) * 128]
                    for c in range(N_CHUNKS):
                        cs = c * CHUNK
                        ps = mmp.tile([128, 512], f32, tag="ps")
                        nc.tensor.matmul(
                            ps[:],
                            lhsT=ones3[:],
                            rhs=negx2sb[:, cs : cs + 512],
                            start=True,
                            stop=False,
                        )
                        ch = xhc[c][:]
                        cl = xlc[c][:]
                        nc.tensor.matmul(
                            ps[:], lhsT=rh, rhs=ch, start=False, stop=False
                        )
                        nc.tensor.matmul(
                            ps[:], lhsT=rh, rhs=cl, start=False, stop=False
                        )
                        nc.tensor.matmul(
                            ps[:], lhsT=rl, rhs=ch, start=False, stop=True
                        )
                        nc.scalar.copy(s_sb[:, cs : cs + 512], ps[:])

                    cand = smallp.tile([128, 8 * N_CHUNKS], f32, tag="cand")
                    for c in range(N_CHUNKS):
                        nc.vector.max(
                            cand[:, c * 8 : (c + 1) * 8],
                            s_sb[:, c * CHUNK : (c + 1) * CHUNK],
                        )
                    f8a = smallp.tile([128, 8], f32, tag="f8a")
                    nc.vector.max(f8a[:], cand[:])
                    cand_mr = smallp.tile([128, 8 * N_CHUNKS], f32, tag="cmr")
                    nc.vector.match_replace(
                        out=cand_mr[:],
                        in_to_replace=f8a[:],
                        in_values=cand[:],
                        imm_value=NEG_INF,
                    )
                    f8b = smallp.tile([128, 8], f32, tag="f8b")
                    nc.vector.max(f8b[:], cand_mr[:])

                    idx16 = smallp.tile([128, K], u32, tag="idx")
                    nc.vector.max_index(idx16[:, 0:8], f8a[:], s_sb[:])
                    nc.vector.max_index(idx16[:, 8:16], f8b[:], s_sb[:])
                    nc.sync.dma_start(
                        idx_out[t * 128 : (t + 1) * 128, :], idx16[:]
                    )

    nc.compile()
    return nc


def get_program():
    global _compiled
    if _compiled is None:
        _compiled = build_program()
    return _compiled


def make_in_maps(x):
    """x: [N_SETS, M, D] float32 -> list of 8 per-core input dicts."""
    x = np.asarray(x, dtype=np.float32)
    in_maps = []
    for c in range(N_CORES):
        s, half = divmod(c, 2)
        xs = x[s]
        if half:
            xs = np.concatenate([xs[ROWS_PER_CORE:], xs[:ROWS_PER_CORE]], axis=0)
        xs = np.ascontiguousarray(xs)
        x2 = np.einsum("md,md->m", xs, xs, dtype=np.float32).astype(np.float32)
        b = (-0.5 * x2).astype(np.float32)
        b3 = np.zeros((3, M), dtype=ml_dtypes.bfloat16)
        rem = b
        for i in range(3):
            b3[i] = rem.astype(ml_dtypes.bfloat16)
            rem = rem - b3[i].astype(np.float32)
        xt = np.ascontiguousarray(xs.T)
        xth = _round_f32r(xt)
        xtl = _round_f32r(xt - xth)
        in_maps.append({"xth": xth, "xtl": xtl, "negx2b3": b3})
    return in_maps


def assemble(per_core_idx):
    """per_core_idx: list of 8 [2048,16] u32 arrays -> (src, dst) int32."""
    src = np.empty((N_SETS, M, K), dtype=np.int64)
    for c in range(N_CORES):
        s, half = divmod(c, 2)
        idx = per_core_idx[c].astype(np.int64)
        local = (idx + half * ROWS_PER_CORE) % M
        src[s, half * ROWS_PER_CORE : (half + 1) * ROWS_PER_CORE, :] = local + s * M
    src = src.reshape(-1).astype(np.int32)
    dst = np.repeat(np.arange(N_SETS * M, dtype=np.int32), K)
    return src, dst


def run_spmd(x, trace=False, **kwargs):
    from concourse import bass_utils

    nc = get_program()
    in_maps = make_in_maps(x)
    res = bass_utils.run_bass_kernel_spmd(
        nc, in_maps, core_ids=list(range(N_CORES)), trace=trace, **kwargs
    )
    per_core = [res.results[c]["idx_out"] for c in range(N_CORES)]
    return assemble(per_core), res


def kernel(x, k):
    k = int(np.asarray(k))
    assert k == K, f"kernel hardcoded for k={K}, got {k}"
    x = np.asarray(x, dtype=np.float32)
    assert x.shape == (N_SETS, M, D), f"unexpected shape {x.shape}"
    (src, dst), _ = run_spmd(x)
    return src, dst
